# revision 1
# baseline (speedup 1.0000x reference)
"""Single-head attention on 8 TRN2 NeuronCores — data-parallel over batch.

Reference (per batch element b):
    q = x @ Wq.T + bq; k = x @ Wk.T + bk; v = x @ Wv.T + bv     [S, D]
    scores = q @ k.T / sqrt(S); masked where attention_mask==0
    out = softmax(scores) @ v                                    [S, D]

Shapes: B=8, S=2048, DIN=1024, D=128.  Core i computes batch element i.

Device-side layout (all host-prepped, bf16 compute / f32 accumulate):
    xT   [128, 8*2048]  xT[p, c*2048+s] = x[s, c*128+p]
    wq/wk/wv [128, 8*128]  w[p, c*128+d] = W[d, c*128+p]   (i.e. W.T chunked)
    scores are built transposed: ST[j, i] = k_j . q_i / sqrt(S); masking is an
    additive -80 bias on masked key rows pre-exp (exp -> ~1e-35 ~= 0).
    context^T[d, i] = sum_j v[j, d] * expT[j, i]; denominators via a ones-column
    matmul; final PE transpose back to [i, d] and a per-partition reciprocal
    multiply.
"""

import numpy as np
import ml_dtypes

B, S, DIN, DOUT = 8, 2048, 1024, 128
N_CORES = 8
NCH = DIN // 128          # 8 contraction chunks
NJT = S // 128            # 16 key tiles
NIC = S // 512            # 4 query chunks of 512
BF16 = ml_dtypes.bfloat16
SCALE = 1.0 / float(np.sqrt(S))

_CACHED = {}


def _build():
    import concourse.bacc as bacc
    import concourse.mybir as mybir
    from concourse.tile import TileContext

    dt = mybir.dt
    F32, BF = dt.float32, dt.bfloat16
    Exp = mybir.ActivationFunctionType.Exp

    nc = bacc.Bacc("TRN2", target_bir_lowering=False)

    xT = nc.declare_dram_parameter("xT", [128, NCH * S], BF, False)
    wq = nc.declare_dram_parameter("wq", [128, NCH * 128], BF, False)
    wk = nc.declare_dram_parameter("wk", [128, NCH * 128], BF, False)
    wv = nc.declare_dram_parameter("wv", [128, NCH * 128], BF, False)
    bq = nc.declare_dram_parameter("bq", [128, 1], F32, False)
    bk = nc.declare_dram_parameter("bk", [128, 1], F32, False)
    bv = nc.declare_dram_parameter("bv", [1, 128], BF, False)
    onesr = nc.declare_dram_parameter("onesr", [1, 128], BF, False)       # row of 1s
    onec = nc.declare_dram_parameter("onec", [128, 1], BF, False)         # col of 1s
    mbias = nc.declare_dram_parameter("mbias", [128, NJT], F32, False)    # 0 / -80
    ident = nc.declare_dram_parameter("ident", [128, 128], F32, False)
    out = nc.declare_dram_parameter("out", [S, DOUT], F32, True)

    with TileContext(nc) as tc:
        with (
            tc.tile_pool(name="const", bufs=1) as cp,
            tc.tile_pool(name="work", bufs=1) as wp,
            tc.tile_pool(name="io", bufs=2) as iop,
        ):
            # ---- warm the exp table while DMAs run ----
            warm = wp.tile([128, 16], F32, tag="warm")
            nc.gpsimd.memset(warm[:], 0.0)
            warm2 = wp.tile([128, 16], F32, tag="warm2")
            nc.scalar.activation(warm2[:], warm[:], Exp)

            # ---- constant loads ----
            xT_sb = cp.tile([128, NCH * S], BF, tag="xT")
            nc.sync.dma_start(out=xT_sb[:], in_=xT[:])
            wq_sb = cp.tile([128, NCH * 128], BF, tag="wq")
            nc.sync.dma_start(out=wq_sb[:], in_=wq[:])
            wk_sb = cp.tile([128, NCH * 128], BF, tag="wk")
            nc.sync.dma_start(out=wk_sb[:], in_=wk[:])
            wv_sb = cp.tile([128, NCH * 128], BF, tag="wv")
            nc.sync.dma_start(out=wv_sb[:], in_=wv[:])
            bq_sb = cp.tile([128, 1], F32, tag="bq")
            nc.sync.dma_start(out=bq_sb[:], in_=bq[:])
            bk_sb = cp.tile([128, 1], F32, tag="bk")
            nc.sync.dma_start(out=bk_sb[:], in_=bk[:])
            bv_sb = cp.tile([1, 128], BF, tag="bv")
            nc.sync.dma_start(out=bv_sb[:], in_=bv[:])
            onesr_sb = cp.tile([1, 128], BF, tag="onesr")
            nc.sync.dma_start(out=onesr_sb[:], in_=onesr[:])
            onec_sb = cp.tile([128, 1], BF, tag="onec")
            nc.sync.dma_start(out=onec_sb[:], in_=onec[:])
            mbias_sb = cp.tile([128, NJT], F32, tag="mbias")
            nc.sync.dma_start(out=mbias_sb[:], in_=mbias[:])
            ident_sb = cp.tile([128, 128], F32, tag="ident")
            nc.sync.dma_start(out=ident_sb[:], in_=ident[:])

            # ---- qT / kT projections: qT[d, s] = sum_din W[d,din] x[s,din] ----
            qT_sb = wp.tile([128, S], BF, tag="qT")
            kT_sb = wp.tile([128, S], BF, tag="kT")
            v_sb = []
            with (
                tc.tile_pool(name="pqk", bufs=2, space="PSUM") as pqk,
                tc.tile_pool(name="pv", bufs=2, space="PSUM") as pv,
            ):
                for w_sb, b_sb, o_sb in ((wq_sb, bq_sb, qT_sb),
                                         (wk_sb, bk_sb, kT_sb)):
                    for h in range(2):  # halves of s
                        ps = pqk.tile([128, 1024], F32, tag="qkpsum", name="qkps")
                        for c in range(NCH):
                            for n in range(2):
                                nc.tensor.matmul(
                                    ps[:, n * 512:(n + 1) * 512],
                                    w_sb[:, c * 128:(c + 1) * 128],
                                    xT_sb[:, c * S + h * 1024 + n * 512:
                                          c * S + h * 1024 + (n + 1) * 512],
                                    start=(c == 0), stop=(c == NCH - 1),
                                )
                        nc.vector.tensor_scalar_add(
                            o_sb[:, h * 1024:(h + 1) * 1024], ps[:], b_sb[:])

                # ---- v projection (natural layout per key tile) + bias ----
                for jt in range(NJT):
                    ps = pv.tile([128, 128], F32, tag="vpsum", name="vps")
                    for c in range(NCH):
                        nc.tensor.matmul(
                            ps[:],
                            xT_sb[:, c * S + jt * 128: c * S + (jt + 1) * 128],
                            wv_sb[:, c * 128:(c + 1) * 128],
                            start=(c == 0), stop=False,
                        )
                    # bias via K=1 outer product: ones[s] x bv[d]
                    nc.tensor.matmul(ps[:], onesr_sb[:], bv_sb[:],
                                     start=False, stop=True)
                    vt = wp.tile([128, 128], BF, tag=f"v{jt}", name=f"v{jt}")
                    nc.vector.tensor_copy(vt[:], ps[:])
                    v_sb.append(vt)

            # ---- scores^T + exp + denominator accumulation ----
            sums_sb = wp.tile([128, S], F32, tag="sums_sb")
            nc.gpsimd.memset(sums_sb[:], 0.0)
            expT_sb = []
            with tc.tile_pool(name="psums", bufs=1, space="PSUM") as psums:
                sums_ps = [
                    psums.tile([1, 512], F32, tag=f"sums{ic}", name=f"sums{ic}")
                    for ic in range(NIC)
                ]
                with tc.tile_pool(name="pS", bufs=2, space="PSUM") as pS:
                    for jt in range(NJT):
                        et = wp.tile([128, S], BF, tag=f"expT{jt}",
                                     name=f"expT{jt}")
                        expT_sb.append(et)
                        for h in range(2):
                            ps = pS.tile([128, 1024], F32, tag="Spsum",
                                         name="Sps")
                            for n in range(2):
                                nc.tensor.matmul(
                                    ps[:, n * 512:(n + 1) * 512],
                                    kT_sb[:, jt * 128:(jt + 1) * 128],
                                    qT_sb[:, h * 1024 + n * 512:
                                          h * 1024 + (n + 1) * 512],
                                    start=True, stop=True,
                                )
                            nc.scalar.activation(
                                et[:, h * 1024:(h + 1) * 1024], ps[:], Exp,
                                bias=mbias_sb[:, jt:jt + 1], scale=SCALE)
                        for ic in range(NIC):
                            nc.tensor.matmul(
                                sums_ps[ic][:],
                                onec_sb[:],
                                et[:, ic * 512:(ic + 1) * 512],
                                start=(jt == 0), stop=(jt == NJT - 1),
                            )

                # ---- denominators out of PSUM ----
                for ic in range(NIC):
                    nc.vector.tensor_copy(
                        sums_sb[0:1, ic * 512:(ic + 1) * 512], sums_ps[ic][:])

            # ---- reciprocal of denominators, transposed to [i_in, itile] ----
            sumsT_sb = wp.tile([128, NJT], F32, tag="sumsT")
            recipT_sb = wp.tile([128, NJT], F32, tag="recipT")
            with (
                tc.tile_pool(name="pctx", bufs=2, space="PSUM") as pctx,
                tc.tile_pool(name="pmisc", bufs=2, space="PSUM") as pmisc,
            ):
                for it in range(NJT):
                    pt = pmisc.tile([128, 128], F32, tag="sTpsum", name="sTps")
                    nc.tensor.transpose(
                        pt[:], sums_sb[:, it * 128:(it + 1) * 128], ident_sb[:])
                    nc.vector.tensor_copy(sumsT_sb[:, it:it + 1], pt[:, 0:1])
                nc.vector.reciprocal(recipT_sb[:], sumsT_sb[:])

                # ---- context^T, transpose back, normalize, store ----
                for ic in range(NIC):
                    pc = pctx.tile([128, 512], F32, tag="ctxpsum", name="ctxps")
                    for jt in range(NJT):
                        nc.tensor.matmul(
                            pc[:],
                            v_sb[jt][:],
                            expT_sb[jt][:, ic * 512:(ic + 1) * 512],
                            start=(jt == 0), stop=(jt == NJT - 1),
                        )
                    ctxT_sb = iop.tile([128, 512], F32, tag="ctxT", name="ctxT")
                    nc.vector.tensor_copy(ctxT_sb[:], pc[:])
                    pn = pmisc.tile([128, 512], F32, tag="natpsum", name="natps")
                    for t in range(4):
                        nc.tensor.transpose(
                            pn[:, t * 128:(t + 1) * 128],
                            ctxT_sb[:, t * 128:(t + 1) * 128], ident_sb[:])
                    o_sb = iop.tile([128, 512], F32, tag="osb", name="osb")
                    for t in range(4):
                        it = ic * 4 + t
                        nc.vector.tensor_scalar_mul(
                            o_sb[:, t * 128:(t + 1) * 128],
                            pn[:, t * 128:(t + 1) * 128],
                            recipT_sb[:, it:it + 1])
                    for t in range(4):
                        r0 = ic * 512 + t * 128
                        nc.sync.dma_start(
                            out=out[r0:r0 + 128, :],
                            in_=o_sb[:, t * 128:(t + 1) * 128])

    nc.compile()
    return nc


def _prep_core_inputs(xb, Wq, bq, Wk, bk, Wv, bv, maskb):
    """Host-side layout prep for one batch element."""
    def chunkT(m):  # [S_or_D, DIN] -> [128, NCH*cols] with m.T chunked over DIN
        mt = np.ascontiguousarray(m.T)          # [DIN, cols]
        c = mt.shape[1]
        return np.ascontiguousarray(
            mt.reshape(NCH, 128, c).transpose(1, 0, 2).reshape(128, NCH * c)
        ).astype(BF16)

    mb = np.where(maskb.reshape(NJT, 128).T != 0, 0.0, -80.0).astype(np.float32)
    return {
        "xT": chunkT(xb),
        "wq": chunkT(Wq), "wk": chunkT(Wk), "wv": chunkT(Wv),
        "bq": bq.reshape(128, 1).astype(np.float32),
        "bk": bk.reshape(128, 1).astype(np.float32),
        "bv": bv.reshape(1, 128).astype(BF16),
        "onesr": np.ones((1, 128), dtype=BF16),
        "onec": np.ones((128, 1), dtype=BF16),
        "mbias": np.ascontiguousarray(mb),
        "ident": np.eye(128, dtype=np.float32),
    }


def kernel(x, Wq, bq, Wk, bk, Wv, bv, attention_mask, _trace=False):
    from concourse.bass_utils import run_bass_kernel_spmd

    x = np.asarray(x, dtype=np.float32)
    Wq = np.asarray(Wq, dtype=np.float32)
    Wk = np.asarray(Wk, dtype=np.float32)
    Wv = np.asarray(Wv, dtype=np.float32)
    bq = np.asarray(bq, dtype=np.float32)
    bk = np.asarray(bk, dtype=np.float32)
    bv = np.asarray(bv, dtype=np.float32)
    mask = np.asarray(attention_mask)

    if "nc" not in _CACHED:
        _CACHED["nc"] = _build()
    nc = _CACHED["nc"]

    in_maps = [
        _prep_core_inputs(x[b], Wq, bq, Wk, bk, Wv, bv, mask[b, 0])
        for b in range(B)
    ]
    res = run_bass_kernel_spmd(
        nc, in_maps, core_ids=list(range(N_CORES)), trace=_trace)
    out = np.stack([res.results[b]["out"] for b in range(B)]).astype(np.float32)
    if _trace:
        _CACHED["exec_time_ns"] = res.exec_time_ns
    return out



# revision 4
# speedup vs baseline: 1.3333x; 1.3333x over previous
"""Single-head attention on 8 TRN2 NeuronCores — data-parallel over batch.

Reference (per batch element b):
    q = x @ Wq.T + bq; k = x @ Wk.T + bk; v = x @ Wv.T + bv     [S, D]
    scores = q @ k.T / sqrt(S); masked where attention_mask==0
    out = softmax(scores) @ v                                    [S, D]

Shapes: B=8, S=2048, DIN=1024, D=128.  Core i computes batch element i.

Key optimizations over the straightforward version:
  * Key compaction: softmax is permutation-invariant over keys and ~half the
    keys are masked out.  Host-side we gather the unmasked keys and pad to a
    fixed M_pad (multiple of 256).  scores/exp/context then run on M_pad keys
    instead of S.  Padded keys get v=0 (zeroed via mask multiply) and are
    excluded from the softmax denominator by using the 0/1 mask column as the
    stationary operand of the denominator matmuls.
  * fp8 (e4m3) x and 16*W inputs halve the input DMA; the 1/16 rescale is
    folded into the bias-apply copies.
  * The middle loop is split into two query halves (i in [0,1024), [1024,2048))
    so the context accumulator fits in 2 PSUM banks and the first half's
    normalize/store tail overlaps the second half's sweep.
  * Denominator matmuls (M=1) for the 2 query chunks of a half are packed into
    different 32-column groups of the PE array via tile_position, sharing one
    PSUM bank across all 4 (half, chunk) pairs.
  * PE warm-up matmuls during the initial DMA engage the HAM clock-gate
    (1.2 -> 2.4 GHz) before the real work starts.
"""

import numpy as np
import ml_dtypes

B, S, DIN, DOUT = 8, 2048, 1024, 128
N_CORES = 8
NCH = DIN // 128          # 8 contraction chunks
BF16 = ml_dtypes.bfloat16
FP8 = ml_dtypes.float8_e4m3fn
SCALE = 1.0 / float(np.sqrt(S))
WSCALE = 16.0             # weights shipped as 16*W in fp8; undone in bias copy

_CACHED = {}


def _blocks(total, maxw=512):
    """Split total columns into blocks of at most maxw."""
    out = []
    off = 0
    while off < total:
        w = min(maxw, total - off)
        out.append((off, w))
        off += w
    return out


def _build(m_pad):
    import concourse.bacc as bacc
    import concourse.mybir as mybir
    from concourse.tile import TileContext

    dt = mybir.dt
    F32, BF, F8 = dt.float32, dt.bfloat16, dt.float8e4
    Exp = mybir.ActivationFunctionType.Exp
    Ident = mybir.ActivationFunctionType.Identity
    Copy = mybir.ActivationFunctionType.Copy

    nkt = m_pad // 128            # key tiles
    kblk = _blocks(m_pad)         # column blocks for the k/v projections

    nc = bacc.Bacc("TRN2", target_bir_lowering=False)

    # xq[p, ((sb*8+c)*512)+s'] = x[sb*512+s', c*128+p], 4 slabs of s
    xq = [nc.declare_dram_parameter(f"xq{sb}", [128, NCH * 512], F8, False)
          for sb in range(4)]
    # xkv[c][p, j'] = x[idx[j'], c*128+p]
    xkv = [nc.declare_dram_parameter(f"xkv{c}", [128, m_pad], BF, False)
           for c in range(NCH)]
    wq = nc.declare_dram_parameter("wq", [128, NCH * 128], F8, False)
    wk = nc.declare_dram_parameter("wk", [128, NCH * 128], BF, False)
    wv = nc.declare_dram_parameter("wv", [128, NCH * 128], BF, False)
    bq = nc.declare_dram_parameter("bq", [128, 1], F32, False)
    bk = nc.declare_dram_parameter("bk", [128, 1], F32, False)
    bvT = nc.declare_dram_parameter("bvT", [128, 1], F32, False)
    mcol = nc.declare_dram_parameter("mcol", [128, nkt], BF, False)
    mcolf = nc.declare_dram_parameter("mcolf", [128, nkt], F32, False)
    identb = nc.declare_dram_parameter("identb", [128, 128], BF, False)
    identf = nc.declare_dram_parameter("identf", [128, 128], F32, False)
    out = nc.declare_dram_parameter("out", [S, DOUT], F32, True)

    with TileContext(nc) as tc:
        with (
            tc.tile_pool(name="const", bufs=1) as cp,
            tc.tile_pool(name="work", bufs=1) as wp,
            tc.tile_pool(name="et", bufs=3) as ep,
            tc.tile_pool(name="ctxs", bufs=2) as xp,
            tc.tile_pool(name="io", bufs=4) as iop,
        ):
            # ---- warm scratch (no DMA dependency) ----
            warm = wp.tile([128, 512], BF, tag="warm")
            nc.gpsimd.memset(warm[:], 0.0)
            warmf = wp.tile([128, 16], F32, tag="warmf")
            nc.gpsimd.memset(warmf[:], 0.0)
            warme = wp.tile([128, 16], F32, tag="warme")
            nc.scalar.activation(warme[:], warmf[:], Exp)   # load exp table

            # ---- input DMAs (program order ~ queue order) ----
            wk_sb = cp.tile([128, NCH * 128], BF, tag="wk")
            nc.sync.dma_start(out=wk_sb[:], in_=wk[:])
            wv_sb = cp.tile([128, NCH * 128], BF, tag="wv")
            nc.sync.dma_start(out=wv_sb[:], in_=wv[:])
            wq_sb = cp.tile([128, NCH * 128], F8, tag="wq")
            nc.sync.dma_start(out=wq_sb[:], in_=wq[:])
            bk_sb = cp.tile([128, 1], F32, tag="bk")
            nc.sync.dma_start(out=bk_sb[:], in_=bk[:])
            bq_sb = cp.tile([128, 1], F32, tag="bq")
            nc.sync.dma_start(out=bq_sb[:], in_=bq[:])
            bvT_sb = cp.tile([128, 1], F32, tag="bvT")
            nc.sync.dma_start(out=bvT_sb[:], in_=bvT[:])
            mcol_sb = cp.tile([128, nkt], BF, tag="mcol")
            nc.sync.dma_start(out=mcol_sb[:], in_=mcol[:])
            mcolf_sb = cp.tile([128, nkt], F32, tag="mcolf")
            nc.sync.dma_start(out=mcolf_sb[:], in_=mcolf[:])
            identb_sb = cp.tile([128, 128], BF, tag="identb")
            nc.sync.dma_start(out=identb_sb[:], in_=identb[:])
            identf_sb = cp.tile([128, 128], F32, tag="identf")
            nc.sync.dma_start(out=identf_sb[:], in_=identf[:])

            xkv_sb = []
            for c in range(NCH):
                t = cp.tile([128, m_pad], BF, tag=f"xkv{c}")
                nc.sync.dma_start(out=t[:], in_=xkv[c][:])
                xkv_sb.append(t)
            xq_sb = []
            for sb in range(4):
                t = cp.tile([128, NCH * 512], F8, tag=f"xq{sb}")
                nc.sync.dma_start(out=t[:], in_=xq[sb][:])
                xq_sb.append(t)

            qT_sb = wp.tile([128, S], BF, tag="qT")
            kT_sb = wp.tile([128, m_pad], BF, tag="kT")
            vT_sb = wp.tile([128, m_pad], BF, tag="vT")
            v_sb = wp.tile([128, m_pad], BF, tag="v")

            # ================= prologue =================
            with (
                tc.tile_pool(name="warmp", bufs=1, space="PSUM") as wpp,
                tc.tile_pool(name="proj", bufs=3, space="PSUM") as pp,
                tc.tile_pool(name="vtp", bufs=2, space="PSUM") as vp,
            ):
                # PE warm-up: engage HAM while DMAs stream
                wps = wpp.tile([128, 512], F32, tag="warmps")
                for _ in range(8):
                    nc.tensor.matmul(wps[:], warm[:, 0:128], warm[:],
                                     start=True, stop=True)

                # ---- kT[d, j'] = sum_c (16Wk)^T x -> *1/16 + bk ----
                kps = [pp.tile([128, w], F32, tag="proj", name=f"kps{i}")
                       for i, (o, w) in enumerate(kblk)]
                for c in range(NCH):
                    for i, (o, w) in enumerate(kblk):
                        nc.tensor.matmul(
                            kps[i][:], wk_sb[:, c * 128:(c + 1) * 128],
                            xkv_sb[c][:, o:o + w],
                            start=(c == 0), stop=(c == NCH - 1))
                for i, (o, w) in enumerate(kblk):
                    nc.scalar.activation(kT_sb[:, o:o + w], kps[i][:], Ident,
                                         bias=bk_sb[:])

                # ---- vT then transpose to natural v, mask-zero pads ----
                vps = [pp.tile([128, w], F32, tag="proj", name=f"vps{i}")
                       for i, (o, w) in enumerate(kblk)]
                for c in range(NCH):
                    for i, (o, w) in enumerate(kblk):
                        nc.tensor.matmul(
                            vps[i][:], wv_sb[:, c * 128:(c + 1) * 128],
                            xkv_sb[c][:, o:o + w],
                            start=(c == 0), stop=(c == NCH - 1))
                for i, (o, w) in enumerate(kblk):
                    nc.vector.tensor_scalar_add(
                        vT_sb[:, o:o + w], vps[i][:], bvT_sb[:])
                for jt in range(nkt):
                    tv = vp.tile([128, 128], BF, tag="vt", name=f"vt{jt}")
                    nc.tensor.transpose(
                        tv[:], vT_sb[:, jt * 128:(jt + 1) * 128], identb_sb[:])
                    nc.vector.tensor_scalar_mul(
                        v_sb[:, jt * 128:(jt + 1) * 128], tv[:],
                        mcolf_sb[:, jt:jt + 1])

                # ---- qT first half (i in [0, 1024)) ----
                qps = [pp.tile([128, 512], F32, tag="proj", name=f"qps{i}")
                       for i in range(2)]
                for c in range(NCH):
                    for sb in range(2):
                        nc.tensor.matmul(
                            qps[sb][:], wq_sb[:, c * 128:(c + 1) * 128],
                            xq_sb[sb][:, c * 512:(c + 1) * 512],
                            start=(c == 0), stop=(c == NCH - 1))
                for sb in range(2):
                    nc.scalar.activation(
                        qT_sb[:, sb * 512:(sb + 1) * 512], qps[sb][:], Ident,
                        bias=bq_sb[:], scale=1.0 / WSCALE)

            # ================= middle sweep =================
            with (
                tc.tile_pool(name="sp", bufs=2, space="PSUM") as sp,
                tc.tile_pool(name="cpp", bufs=1, space="PSUM") as cpp,
                tc.tile_pool(name="sums", bufs=1, space="PSUM") as smp,
                tc.tile_pool(name="tp", bufs=1, space="PSUM") as tpp,
            ):
                sums_ps = smp.tile([128, 512], F32, tag="sums")
                ss_sb = wp.tile([128, 512], F32, tag="ss")
                rt_sb = wp.tile([128, 2 * 8], F32, tag="rt")
                recip_sb = wp.tile([128, 2 * 8], F32, tag="recip")

                def qh1_chunk(sb):
                    # second-half q projection, borrowing the tp bank
                    qp = tpp.tile([128, 512], F32, tag="tp", name=f"qh1_{sb}")
                    for c in range(NCH):
                        nc.tensor.matmul(
                            qp[:], wq_sb[:, c * 128:(c + 1) * 128],
                            xq_sb[sb][:, c * 512:(c + 1) * 512],
                            start=(c == 0), stop=(c == NCH - 1))
                    nc.scalar.activation(
                        qT_sb[:, sb * 512:(sb + 1) * 512], qp[:], Ident,
                        bias=bq_sb[:], scale=1.0 / WSCALE)

                def tail(h, ctx_ps):
                    # denominators: psum rows {64h, 64h+32} -> recip_sb cols
                    nc.vector.tensor_copy(
                        ss_sb[64 * h:64 * h + 64, :],
                        sums_ps[64 * h:64 * h + 64, :])
                    for b in range(4):
                        ts = tpp.tile([128, 128], F32, tag="tp",
                                      name=f"sT{h}_{b}")
                        nc.tensor.transpose(
                            ts[:], ss_sb[:, b * 128:(b + 1) * 128],
                            identf_sb[:])
                        for icg in range(2):
                            it = h * 8 + icg * 4 + b
                            nc.vector.tensor_copy(
                                rt_sb[:, it:it + 1],
                                ts[:, 64 * h + 32 * icg:64 * h + 32 * icg + 1])
                    nc.vector.reciprocal(
                        recip_sb[:, h * 8:(h + 1) * 8],
                        rt_sb[:, h * 8:(h + 1) * 8])
                    # context: copy out of psum, transpose, normalize, store
                    ctx_sb = xp.tile([128, 1024], F32, tag="ctxT",
                                     name=f"ctxT{h}")
                    nc.vector.tensor_copy(ctx_sb[:], ctx_ps[:])
                    for icg in range(2):
                        o_sb = iop.tile([128, 512], F32, tag="osb",
                                        name=f"osb{h}_{icg}")
                        for b in range(4):
                            tc2 = tpp.tile([128, 128], F32, tag="tp",
                                           name=f"cT{h}_{icg}_{b}")
                            nc.tensor.transpose(
                                tc2[:],
                                ctx_sb[:, icg * 512 + b * 128:
                                       icg * 512 + (b + 1) * 128],
                                identf_sb[:])
                            it = h * 8 + icg * 4 + b
                            nc.scalar.activation(
                                o_sb[:, b * 128:(b + 1) * 128], tc2[:], Copy,
                                scale=recip_sb[:, it:it + 1])
                        for b in range(4):
                            r0 = h * 1024 + icg * 512 + b * 128
                            nc.sync.dma_start(
                                out=out[r0:r0 + 128, :],
                                in_=o_sb[:, b * 128:(b + 1) * 128])

                prev_ctx = None
                for h in range(2):
                    ctx_ps = cpp.tile([128, 1024], F32, tag="ctx",
                                      name=f"ctx{h}")
                    for jt in range(nkt):
                        # scores^T for this key tile against the half's queries
                        ps = sp.tile([128, 1024], F32, tag="sc",
                                     name=f"sc{h}_{jt}")
                        for n in range(2):
                            nc.tensor.matmul(
                                ps[:, n * 512:(n + 1) * 512],
                                kT_sb[:, jt * 128:(jt + 1) * 128],
                                qT_sb[:, h * 1024 + n * 512:
                                      h * 1024 + (n + 1) * 512],
                                start=True, stop=True)
                        et = ep.tile([128, 1024], BF, tag="et",
                                     name=f"et{h}_{jt}")
                        nc.scalar.activation(et[:], ps[:], Exp, scale=SCALE)
                        # denominator (mask column excludes padded keys)
                        for icg in range(2):
                            base = 64 * h + 32 * icg
                            nc.tensor.matmul(
                                sums_ps[base:base + 1, :],
                                mcol_sb[:, jt:jt + 1],
                                et[:, icg * 512:(icg + 1) * 512],
                                start=(jt == 0), stop=(jt == nkt - 1),
                                tile_position=(0, base),
                                skip_group_check=True)
                        # context accumulation
                        for n in range(2):
                            nc.tensor.matmul(
                                ctx_ps[:, n * 512:(n + 1) * 512],
                                v_sb[:, jt * 128:(jt + 1) * 128],
                                et[:, n * 512:(n + 1) * 512],
                                start=(jt == 0), stop=(jt == nkt - 1))
                        if h == 0 and jt == 2:
                            qh1_chunk(2)
                        elif h == 0 and jt == 4:
                            qh1_chunk(3)
                    if h == 0:
                        prev_ctx = ctx_ps
                        tail(0, prev_ctx)
                tail(1, ctx_ps)

    nc.compile()
    return nc


def _prep_core_inputs(xb, Wq, bq, Wk, bk, Wv, bv, maskb, m_pad):
    """Host-side layout prep for one batch element."""
    nkt = m_pad // 128
    idx = np.nonzero(maskb != 0)[0]
    cnt = len(idx)

    # gathered keys, padded with zeros
    xg = np.zeros((m_pad, DIN), dtype=np.float32)
    xg[:cnt] = xb[idx]

    def chunkT(m, dt):  # [rows, DIN] -> [128, NCH*rows], W.T chunked over DIN
        mt = np.ascontiguousarray(m.T)          # [DIN, rows]
        c = mt.shape[1]
        return np.ascontiguousarray(
            mt.reshape(NCH, 128, c).transpose(1, 0, 2).reshape(128, NCH * c)
        ).astype(dt)

    # xq slabs: [128, (sb,c,s')] ; xq4[p, (sb*8+c)*512+s'] = x[sb*512+s', c*128+p]
    xt = xb.T.reshape(NCH, 128, 4, 512)          # [c, p, sb, s']
    xq4 = np.ascontiguousarray(
        xt.transpose(1, 2, 0, 3).reshape(128, 4 * NCH * 512)).astype(FP8)

    # xkv per chunk: [128, m_pad]
    xkvt = xg.T.reshape(NCH, 128, m_pad)         # [c, p, j']
    xkv = [np.ascontiguousarray(xkvt[c]).astype(BF16) for c in range(NCH)]

    m01 = np.zeros((m_pad,), dtype=np.float32)
    m01[:cnt] = 1.0
    mt = m01.reshape(nkt, 128).T                 # [128, nkt]

    d = {
        "wq": chunkT(Wq * WSCALE, FP8),
        "wk": chunkT(Wk, BF16),
        "wv": chunkT(Wv, BF16),
        "bq": bq.reshape(128, 1).astype(np.float32),
        "bk": bk.reshape(128, 1).astype(np.float32),
        "bvT": bv.reshape(128, 1).astype(np.float32),
        "mcol": np.ascontiguousarray(mt).astype(BF16),
        "mcolf": np.ascontiguousarray(mt),
        "identb": np.eye(128, dtype=np.float32).astype(BF16),
        "identf": np.eye(128, dtype=np.float32),
    }
    for sb in range(4):
        d[f"xq{sb}"] = np.ascontiguousarray(
            xq4[:, sb * NCH * 512:(sb + 1) * NCH * 512])
    for c in range(NCH):
        d[f"xkv{c}"] = xkv[c]
    return d


def kernel(x, Wq, bq, Wk, bk, Wv, bv, attention_mask, _trace=False):
    from concourse.bass_utils import run_bass_kernel_spmd

    x = np.asarray(x, dtype=np.float32)
    Wq = np.asarray(Wq, dtype=np.float32)
    Wk = np.asarray(Wk, dtype=np.float32)
    Wv = np.asarray(Wv, dtype=np.float32)
    bq = np.asarray(bq, dtype=np.float32)
    bk = np.asarray(bk, dtype=np.float32)
    bv = np.asarray(bv, dtype=np.float32)
    mask = np.asarray(attention_mask)

    counts = [int((mask[b, 0] != 0).sum()) for b in range(B)]
    m_pad = min(S, max(512, int(-(-max(counts) // 256) * 256)))

    if ("nc", m_pad) not in _CACHED:
        _CACHED[("nc", m_pad)] = _build(m_pad)
    nc = _CACHED[("nc", m_pad)]

    in_maps = [
        _prep_core_inputs(x[b], Wq, bq, Wk, bk, Wv, bv, mask[b, 0], m_pad)
        for b in range(B)
    ]
    res = run_bass_kernel_spmd(
        nc, in_maps, core_ids=list(range(N_CORES)), trace=_trace)
    out = np.stack([res.results[b]["out"] for b in range(B)]).astype(np.float32)
    if _trace:
        _CACHED["exec_time_ns"] = res.exec_time_ns
    return out


# revision 7
# speedup vs baseline: 1.4542x; 1.0907x over previous
"""Single-head attention on 8 TRN2 NeuronCores — data-parallel over batch.

Reference (per batch element b):
    q = x @ Wq.T + bq; k = x @ Wk.T + bk; v = x @ Wv.T + bv     [S, D]
    scores = q @ k.T / sqrt(S); masked where attention_mask==0
    out = softmax(scores) @ v                                    [S, D]

Shapes: B=8, S=2048, DIN=1024, D=128.  Core i computes batch element i.

Key optimizations over the straightforward version:
  * Key compaction: softmax is permutation-invariant over keys and ~half the
    keys are masked out.  Host-side we gather the unmasked keys and pad to a
    fixed M_pad (multiple of 128).  scores/exp/context then run on M_pad keys
    instead of S.  Padded keys get v=0 (zeroed via mask multiply) and are
    excluded from the softmax denominator by using the 0/1 mask column as the
    stationary operand of the denominator matmuls.
  * fp8 (e4m3) x and 16*Wq for the q projection halve that DMA; the 1/16
    rescale is folded into the bias-apply copy.  The k/v path stays bf16 —
    the near-uniform softmax averages ~1000 v rows with heavy cancellation,
    so fp8 noise in v does not shrink relative to the output.
  * Input DMAs are spread across the sync/gpsimd/vector/scalar queues and
    sized large, since each dma_start costs ~0.7us of issue time on its queue.
  * The middle loop is split into two query halves (i in [0,1024), [1024,2048))
    so the context accumulator fits in 2 PSUM banks and the first half's
    normalize/store tail overlaps the second half's sweep.  Consumer matmuls
    (denominator + context) of step jt are emitted after scores of step jt+1
    so the in-order PE never idles waiting for exp.
  * Denominator matmuls (M=1) for the 2 query chunks of a half are packed into
    different 32-column groups of the PE array via tile_position, sharing one
    PSUM bank across all 4 (half, chunk) pairs.
  * PE warm-up matmuls during the initial DMA engage the HAM clock-gate
    (1.2 -> 2.4 GHz) before the real work starts; more are interleaved into
    the DMA-paced k projection to keep it engaged.
"""

import numpy as np
import ml_dtypes

B, S, DIN, DOUT = 8, 2048, 1024, 128
N_CORES = 8
NCH = DIN // 128          # 8 contraction chunks
BF16 = ml_dtypes.bfloat16
FP8 = ml_dtypes.float8_e4m3fn
SCALE = 1.0 / float(np.sqrt(S))
WSCALE = 16.0             # Wq shipped as 16*Wq in fp8; undone in bias copy

_CACHED = {}


def _blocks(total, maxw=512):
    out = []
    off = 0
    while off < total:
        w = min(maxw, total - off)
        out.append((off, w))
        off += w
    return out


def _build(m_pad):
    import concourse.bacc as bacc
    import concourse.mybir as mybir
    from concourse.tile import TileContext

    dt = mybir.dt
    F32, BF, F8 = dt.float32, dt.bfloat16, dt.float8e4
    Exp = mybir.ActivationFunctionType.Exp
    Ident = mybir.ActivationFunctionType.Identity
    Copy = mybir.ActivationFunctionType.Copy
    MUL, ADD = mybir.AluOpType.mult, mybir.AluOpType.add

    nkt = m_pad // 128            # key tiles
    kblk = _blocks(m_pad)         # column blocks for the k/v projections
    nb32 = 3 + nkt + 128          # blob32: bq | bk | bvT | mcolf | identf
    nb16 = nkt + 128              # blob16: mcol | identb

    nc = bacc.Bacc("TRN2", target_bir_lowering=False)

    # xq[p, ((sb*8+c)*512)+s'] = x[sb*512+s', c*128+p], 4 slabs of s
    xq = [nc.declare_dram_parameter(f"xq{sb}", [128, NCH * 512], F8, False)
          for sb in range(4)]
    # xkv2[p2][p, i*m_pad + j'] = x[idx[j'], (2*p2+i)*128+p]
    xkv = [nc.declare_dram_parameter(f"xkv{p2}", [128, 2 * m_pad], BF, False)
           for p2 in range(4)]
    wkv = nc.declare_dram_parameter("wkv", [128, 2 * NCH * 128], BF, False)
    wq = nc.declare_dram_parameter("wq", [128, NCH * 128], F8, False)
    blob32 = nc.declare_dram_parameter("blob32", [128, nb32], F32, False)
    blob16 = nc.declare_dram_parameter("blob16", [128, nb16], BF, False)
    out = nc.declare_dram_parameter("out", [S, DOUT], F32, True)

    with TileContext(nc) as tc:
        with (
            tc.tile_pool(name="const", bufs=1) as cp,
            tc.tile_pool(name="work", bufs=1) as wp,
            tc.tile_pool(name="et", bufs=3) as ep,
            tc.tile_pool(name="ctxs", bufs=2) as xp,
            tc.tile_pool(name="io", bufs=4) as iop,
        ):
            # ---- warm scratch (no DMA dependency) ----
            warm = wp.tile([128, 512], BF, tag="warm")
            nc.gpsimd.memset(warm[:], 0.0)
            warmf = wp.tile([128, 16], F32, tag="warmf")
            nc.gpsimd.memset(warmf[:], 0.0)
            warme = wp.tile([128, 16], F32, tag="warme")
            nc.scalar.activation(warme[:], warmf[:], Exp)   # load exp table

            # ---- input DMAs, spread across queues ----
            # gpsimd queue: the k/v-side activations (first thing PE needs)
            xkv_sb = []
            for p2 in range(4):
                t = cp.tile([128, 2 * m_pad], BF, tag=f"xkv{p2}")
                nc.gpsimd.dma_start(out=t[:], in_=xkv[p2][:])
                xkv_sb.append(t)
            # scalar queue: first q half (ACT idle until the k bias copies)
            xq_sb = [None] * 4
            for sb in (0, 1):
                t = cp.tile([128, NCH * 512], F8, tag=f"xq{sb}")
                nc.scalar.dma_start(out=t[:], in_=xq[sb][:])
                xq_sb[sb] = t
            # sync queue: weights, const blobs, second q half
            wkv_sb = cp.tile([128, 2 * NCH * 128], BF, tag="wkv")
            nc.sync.dma_start(out=wkv_sb[:], in_=wkv[:])
            wq_sb = cp.tile([128, NCH * 128], F8, tag="wq")
            nc.sync.dma_start(out=wq_sb[:], in_=wq[:])
            b16_sb = cp.tile([128, nb16], BF, tag="b16")
            nc.sync.dma_start(out=b16_sb[:], in_=blob16[:])
            b32_sb = cp.tile([128, nb32], F32, tag="b32")
            nc.sync.dma_start(out=b32_sb[:], in_=blob32[:])
            for sb in (2, 3):
                t = cp.tile([128, NCH * 512], F8, tag=f"xq{sb}")
                nc.sync.dma_start(out=t[:], in_=xq[sb][:])
                xq_sb[sb] = t

            wk_sb = wkv_sb[:, 0:NCH * 128]
            wv_sb = wkv_sb[:, NCH * 128:2 * NCH * 128]
            bq_sb = b32_sb[:, 0:1]
            bk_sb = b32_sb[:, 1:2]
            bvT_sb = b32_sb[:, 2:3]
            mcolf_sb = b32_sb[:, 3:3 + nkt]
            identf_sb = b32_sb[:, 3 + nkt:3 + nkt + 128]
            mcol_sb = b16_sb[:, 0:nkt]
            identb_sb = b16_sb[:, nkt:nkt + 128]

            qT_sb = wp.tile([128, S], BF, tag="qT")
            kT_sb = wp.tile([128, m_pad], BF, tag="kT")
            vT_sb = wp.tile([128, m_pad], BF, tag="vT")
            v_sb = wp.tile([128, m_pad], BF, tag="v")

            def xkv_ap(c, o, w):
                return xkv_sb[c // 2][:, (c % 2) * m_pad + o:
                                      (c % 2) * m_pad + o + w]

            # ================= prologue =================
            with (
                tc.tile_pool(name="warmp", bufs=1, space="PSUM") as wpp,
                tc.tile_pool(name="proj", bufs=3, space="PSUM") as pp,
                tc.tile_pool(name="vtp", bufs=2, space="PSUM") as vp,
            ):
                # PE warm-up: engage HAM while DMAs stream
                wps = wpp.tile([128, 512], F32, tag="warmps")
                for _ in range(4):
                    nc.tensor.matmul(wps[:], warm[:, 0:128], warm[:],
                                     start=True, stop=True)

                # ---- kT[d, j'] projection (DMA-paced; warm MMs between) ----
                kps = [pp.tile([128, w], F32, tag="proj", name=f"kps{i}")
                       for i, (o, w) in enumerate(kblk)]
                for c in range(NCH):
                    for i, (o, w) in enumerate(kblk):
                        nc.tensor.matmul(
                            kps[i][:], wk_sb[:, c * 128:(c + 1) * 128],
                            xkv_ap(c, o, w),
                            start=(c == 0), stop=(c == NCH - 1))
                    if c % 2 == 0:
                        nc.tensor.matmul(wps[:], warm[:, 0:128], warm[:],
                                         start=True, stop=True)
                for i, (o, w) in enumerate(kblk):
                    nc.scalar.activation(kT_sb[:, o:o + w], kps[i][:], Ident,
                                         bias=bk_sb)

                # ---- qT first half (i in [0, 1024)) ----
                qps = [pp.tile([128, 512], F32, tag="proj", name=f"qps{i}")
                       for i in range(2)]
                for c in range(NCH):
                    for sb in range(2):
                        nc.tensor.matmul(
                            qps[sb][:], wq_sb[:, c * 128:(c + 1) * 128],
                            xq_sb[sb][:, c * 512:(c + 1) * 512],
                            start=(c == 0), stop=(c == NCH - 1))
                for sb in range(2):
                    nc.scalar.activation(
                        qT_sb[:, sb * 512:(sb + 1) * 512], qps[sb][:], Ident,
                        bias=bq_sb, scale=1.0 / WSCALE)

                # ---- vT then transpose to natural v, mask-zero pads ----
                vps = [pp.tile([128, w], F32, tag="proj", name=f"vps{i}")
                       for i, (o, w) in enumerate(kblk)]
                for c in range(NCH):
                    for i, (o, w) in enumerate(kblk):
                        nc.tensor.matmul(
                            vps[i][:], wv_sb[:, c * 128:(c + 1) * 128],
                            xkv_ap(c, o, w),
                            start=(c == 0), stop=(c == NCH - 1))
                for i, (o, w) in enumerate(kblk):
                    nc.vector.tensor_scalar_add(
                        vT_sb[:, o:o + w], vps[i][:], bvT_sb)
                for jt in range(nkt):
                    tv = vp.tile([128, 128], BF, tag="vt", name=f"vt{jt}")
                    nc.tensor.transpose(
                        tv[:], vT_sb[:, jt * 128:(jt + 1) * 128], identb_sb)
                    nc.vector.tensor_scalar_mul(
                        v_sb[:, jt * 128:(jt + 1) * 128], tv[:],
                        mcolf_sb[:, jt:jt + 1])

            # ================= middle sweep =================
            with (
                tc.tile_pool(name="sp", bufs=2, space="PSUM") as sp,
                tc.tile_pool(name="cpp", bufs=1, space="PSUM") as cpp,
                tc.tile_pool(name="sums", bufs=1, space="PSUM") as smp,
                tc.tile_pool(name="tp", bufs=1, space="PSUM") as tpp,
            ):
                sums_ps = smp.tile([128, 512], F32, tag="sums")
                ss_sb = wp.tile([128, 512], F32, tag="ss")
                rt_sb = wp.tile([128, 2 * 8], F32, tag="rt")
                recip_sb = wp.tile([128, 2 * 8], F32, tag="recip")

                def qh1_chunk(sb):
                    # second-half q projection, borrowing the tp bank;
                    # bias copy on DVE so ACT keeps streaming exps
                    qp = tpp.tile([128, 512], F32, tag="tp", name=f"qh1_{sb}")
                    for c in range(NCH):
                        nc.tensor.matmul(
                            qp[:], wq_sb[:, c * 128:(c + 1) * 128],
                            xq_sb[sb][:, c * 512:(c + 1) * 512],
                            start=(c == 0), stop=(c == NCH - 1))
                    nc.vector.tensor_scalar(
                        qT_sb[:, sb * 512:(sb + 1) * 512], qp[:],
                        1.0 / WSCALE, bq_sb, MUL, ADD)

                def emit_scores(h, jt):
                    ps = sp.tile([128, 1024], F32, tag="sc", name=f"sc{h}_{jt}")
                    for n in range(2):
                        nc.tensor.matmul(
                            ps[:, n * 512:(n + 1) * 512],
                            kT_sb[:, jt * 128:(jt + 1) * 128],
                            qT_sb[:, h * 1024 + n * 512:
                                  h * 1024 + (n + 1) * 512],
                            start=True, stop=True)
                    et = ep.tile([128, 1024], BF, tag="et", name=f"et{h}_{jt}")
                    nc.scalar.activation(et[:], ps[:], Exp, scale=SCALE)
                    return et

                def emit_consume(h, jt, et, ctx_ps):
                    for icg in range(2):
                        base = 64 * h + 32 * icg
                        nc.tensor.matmul(
                            sums_ps[base:base + 1, :],
                            mcol_sb[:, jt:jt + 1],
                            et[:, icg * 512:(icg + 1) * 512],
                            start=(jt == 0), stop=(jt == nkt - 1),
                            tile_position=(0, base),
                            skip_group_check=True)
                    for n in range(2):
                        nc.tensor.matmul(
                            ctx_ps[:, n * 512:(n + 1) * 512],
                            v_sb[:, jt * 128:(jt + 1) * 128],
                            et[:, n * 512:(n + 1) * 512],
                            start=(jt == 0), stop=(jt == nkt - 1))

                def tail(h, ctx_ps):
                    # transposes go to the tp bank for h=0 (sweep still needs
                    # sp), and to freed sp slots for h=1 (no bank reuse stalls)
                    if h == 0:
                        tquads = [(tpp.tile([128, 512], F32, tag="tp",
                                            name="t0a"), 0)] * 3
                        tquads = [tquads[0], tquads[1], tquads[2]]
                    else:
                        ta = sp.tile([128, 1024], F32, tag="sc", name="t1a")
                        tb = sp.tile([128, 1024], F32, tag="sc", name="t1b")
                        tquads = [(ta, 0), (ta, 4), (tb, 0)]

                    # denominators: sums psum rows {64h, 64h+32}
                    nc.vector.tensor_copy(
                        ss_sb[64 * h:64 * h + 64, :],
                        sums_ps[64 * h:64 * h + 64, :])
                    ctx_sb = xp.tile([128, 1024], F32, tag="ctxT",
                                     name=f"ctxT{h}")
                    nc.vector.tensor_copy(ctx_sb[:, 0:512], ctx_ps[:, 0:512])

                    stile, soff = tquads[0]
                    for b in range(4):
                        tcol = (soff + b) * 128
                        nc.tensor.transpose(
                            stile[:, tcol:tcol + 128],
                            ss_sb[:, b * 128:(b + 1) * 128], identf_sb)
                        for icg in range(2):
                            it = h * 8 + icg * 4 + b
                            src = tcol + 64 * h + 32 * icg
                            nc.vector.tensor_copy(
                                rt_sb[:, it:it + 1], stile[:, src:src + 1])
                    nc.vector.reciprocal(
                        recip_sb[:, h * 8:(h + 1) * 8],
                        rt_sb[:, h * 8:(h + 1) * 8])

                    nc.vector.tensor_copy(ctx_sb[:, 512:1024],
                                          ctx_ps[:, 512:1024])
                    for icg in range(2):
                        ctile, coff = tquads[1 + icg]
                        o_sb = iop.tile([128, 512], F32, tag="osb",
                                        name=f"osb{h}_{icg}")
                        for b in range(4):
                            tcol = (coff + b) * 128
                            nc.tensor.transpose(
                                ctile[:, tcol:tcol + 128],
                                ctx_sb[:, icg * 512 + b * 128:
                                       icg * 512 + (b + 1) * 128], identf_sb)
                            it = h * 8 + icg * 4 + b
                            if h == 1 and b % 2 == 0:
                                nc.scalar.activation(
                                    o_sb[:, b * 128:(b + 1) * 128],
                                    ctile[:, tcol:tcol + 128], Copy,
                                    scale=recip_sb[:, it:it + 1])
                            else:
                                nc.vector.tensor_scalar_mul(
                                    o_sb[:, b * 128:(b + 1) * 128],
                                    ctile[:, tcol:tcol + 128],
                                    recip_sb[:, it:it + 1])
                        for b in range(4):
                            r0 = h * 1024 + icg * 512 + b * 128
                            eng = nc.sync if b % 2 == 0 else nc.gpsimd
                            eng.dma_start(
                                out=out[r0:r0 + 128, :],
                                in_=o_sb[:, b * 128:(b + 1) * 128])

                ctx_tiles = {}

                def ctx_tile(h):
                    if h not in ctx_tiles:
                        ctx_tiles[h] = cpp.tile([128, 1024], F32, tag="ctx",
                                                name=f"ctx{h}")
                    return ctx_tiles[h]

                for h in range(2):
                    prev = None
                    for jt in range(nkt):
                        et = emit_scores(h, jt)
                        if h == 1 and jt == 1:
                            # h0 tail here: its PE transposes hide in the
                            # exp pipeline-fill bubble of the h1 sweep
                            tail(0, ctx_tile(0))
                        if prev is not None:
                            emit_consume(h, jt - 1, prev, ctx_tile(h))
                        prev = et
                        if h == 0 and jt == 2:
                            qh1_chunk(2)
                        elif h == 0 and jt == 4:
                            qh1_chunk(3)
                    emit_consume(h, nkt - 1, prev, ctx_tile(h))
                tail(1, ctx_tile(1))

    nc.compile()
    return nc


def _prep_core_inputs(xb, Wq, bq, Wk, bk, Wv, bv, maskb, m_pad):
    """Host-side layout prep for one batch element."""
    nkt = m_pad // 128
    idx = np.nonzero(maskb != 0)[0]
    cnt = len(idx)

    xg = np.zeros((m_pad, DIN), dtype=np.float32)
    xg[:cnt] = xb[idx]

    def chunkT(m, dtp):  # [rows, DIN] -> [128, NCH*rows], m.T chunked over DIN
        mt = np.ascontiguousarray(m.T)          # [DIN, rows]
        c = mt.shape[1]
        return np.ascontiguousarray(
            mt.reshape(NCH, 128, c).transpose(1, 0, 2).reshape(128, NCH * c)
        ).astype(dtp)

    xt = xb.T.reshape(NCH, 128, 4, 512)          # [c, p, sb, s']
    xq4 = np.ascontiguousarray(
        xt.transpose(1, 2, 0, 3).reshape(128, 4 * NCH * 512)).astype(FP8)

    xkvt = xg.T.reshape(NCH, 128, m_pad)         # [c, p, j']
    m01 = np.zeros((m_pad,), dtype=np.float32)
    m01[:cnt] = 1.0
    mt = np.ascontiguousarray(m01.reshape(nkt, 128).T)   # [128, nkt]

    identf = np.eye(128, dtype=np.float32)
    blob32 = np.concatenate([
        bq.reshape(128, 1), bk.reshape(128, 1), bv.reshape(128, 1),
        mt, identf], axis=1).astype(np.float32)
    blob16 = np.concatenate([mt, identf], axis=1).astype(BF16)

    d = {
        "wq": chunkT(Wq * WSCALE, FP8),
        "wkv": np.concatenate([chunkT(Wk, BF16), chunkT(Wv, BF16)], axis=1),
        "blob32": np.ascontiguousarray(blob32),
        "blob16": np.ascontiguousarray(blob16),
    }
    for sb in range(4):
        d[f"xq{sb}"] = np.ascontiguousarray(
            xq4[:, sb * NCH * 512:(sb + 1) * NCH * 512])
    for p2 in range(4):
        d[f"xkv{p2}"] = np.ascontiguousarray(
            np.concatenate([xkvt[2 * p2], xkvt[2 * p2 + 1]], axis=1)
        ).astype(BF16)
    return d


def kernel(x, Wq, bq, Wk, bk, Wv, bv, attention_mask, _trace=False):
    from concourse.bass_utils import run_bass_kernel_spmd

    x = np.asarray(x, dtype=np.float32)
    Wq = np.asarray(Wq, dtype=np.float32)
    Wk = np.asarray(Wk, dtype=np.float32)
    Wv = np.asarray(Wv, dtype=np.float32)
    bq = np.asarray(bq, dtype=np.float32)
    bk = np.asarray(bk, dtype=np.float32)
    bv = np.asarray(bv, dtype=np.float32)
    mask = np.asarray(attention_mask)

    counts = [int((mask[b, 0] != 0).sum()) for b in range(B)]
    m_pad = min(S, max(512, int(-(-max(counts) // 128) * 128)))

    if ("nc", m_pad) not in _CACHED:
        _CACHED[("nc", m_pad)] = _build(m_pad)
    nc = _CACHED[("nc", m_pad)]

    in_maps = [
        _prep_core_inputs(x[b], Wq, bq, Wk, bk, Wv, bv, mask[b, 0], m_pad)
        for b in range(B)
    ]
    res = run_bass_kernel_spmd(
        nc, in_maps, core_ids=list(range(N_CORES)), trace=_trace)
    out = np.stack([res.results[b]["out"] for b in range(B)]).astype(np.float32)
    if _trace:
        _CACHED["exec_time_ns"] = res.exec_time_ns
    return out


# revision 8
# speedup vs baseline: 1.4949x; 1.0280x over previous
"""Single-head attention on 8 TRN2 NeuronCores — data-parallel over batch.

Reference (per batch element b):
    q = x @ Wq.T + bq; k = x @ Wk.T + bk; v = x @ Wv.T + bv     [S, D]
    scores = q @ k.T / sqrt(S); masked where attention_mask==0
    out = softmax(scores) @ v                                    [S, D]

Shapes: B=8, S=2048, DIN=1024, D=128.  Core i computes batch element i.

Key optimizations over the straightforward version:
  * Key compaction: softmax is permutation-invariant over keys and ~half the
    keys are masked out.  Host-side we gather the unmasked keys and pad to a
    fixed M_pad (multiple of 128).  scores/exp/context then run on M_pad keys
    instead of S.  Padded keys get v=0 (zeroed via mask multiply) and are
    excluded from the softmax denominator by using the 0/1 mask column as the
    stationary operand of the denominator matmuls.
  * fp8 (e4m3) x and 16*Wq for the q projection halve that DMA; the 1/16
    rescale is folded into the bias-apply copy.  The k/v path stays bf16 —
    the near-uniform softmax averages ~1000 v rows with heavy cancellation,
    so fp8 noise in v does not shrink relative to the output.
  * Input DMAs are spread across the sync/gpsimd/vector/scalar queues and
    sized large, since each dma_start costs ~0.7us of issue time on its queue.
  * The middle loop is split into two query halves (i in [0,1024), [1024,2048))
    so the context accumulator fits in 2 PSUM banks and the first half's
    normalize/store tail overlaps the second half's sweep.  Consumer matmuls
    (denominator + context) of step jt are emitted after scores of step jt+1
    so the in-order PE never idles waiting for exp.
  * Denominator matmuls (M=1) for the 2 query chunks of a half are packed into
    different 32-column groups of the PE array via tile_position, sharing one
    PSUM bank across all 4 (half, chunk) pairs.
  * PE warm-up matmuls during the initial DMA engage the HAM clock-gate
    (1.2 -> 2.4 GHz) before the real work starts; more are interleaved into
    the DMA-paced k projection to keep it engaged.
"""

import numpy as np
import ml_dtypes

B, S, DIN, DOUT = 8, 2048, 1024, 128
N_CORES = 8
NCH = DIN // 128          # 8 contraction chunks
BF16 = ml_dtypes.bfloat16
FP8 = ml_dtypes.float8_e4m3fn
SCALE = 1.0 / float(np.sqrt(S))
WSCALE = 16.0             # Wq shipped as 16*Wq in fp8; undone in bias copy

_CACHED = {}


def _blocks(total, maxw=512):
    out = []
    off = 0
    while off < total:
        w = min(maxw, total - off)
        out.append((off, w))
        off += w
    return out


def _build(m_pad):
    import concourse.bacc as bacc
    import concourse.mybir as mybir
    from concourse.tile import TileContext

    dt = mybir.dt
    F32, BF, F8 = dt.float32, dt.bfloat16, dt.float8e4
    Exp = mybir.ActivationFunctionType.Exp
    Ident = mybir.ActivationFunctionType.Identity
    Copy = mybir.ActivationFunctionType.Copy
    MUL, ADD = mybir.AluOpType.mult, mybir.AluOpType.add

    nkt = m_pad // 128            # key tiles
    kblk = _blocks(m_pad)         # column blocks for the k/v projections
    nb32 = 3 + nkt + 128          # blob32: bq | bk | bvT | mcolf | identf
    nb16 = nkt + 128              # blob16: mcol | identb

    nc = bacc.Bacc("TRN2", target_bir_lowering=False)

    # xq[p, ((sb*8+c)*512)+s'] = x[sb*512+s', c*128+p], 4 slabs of s
    xq = [nc.declare_dram_parameter(f"xq{sb}", [128, NCH * 512], F8, False)
          for sb in range(4)]
    # xkv2[p2][p, i*m_pad + j'] = x[idx[j'], (2*p2+i)*128+p]
    xkv = [nc.declare_dram_parameter(f"xkv{p2}", [128, 2 * m_pad], BF, False)
           for p2 in range(4)]
    wkv = nc.declare_dram_parameter("wkv", [128, 2 * NCH * 128], BF, False)
    wq = nc.declare_dram_parameter("wq", [128, NCH * 128], F8, False)
    blob32 = nc.declare_dram_parameter("blob32", [128, nb32], F32, False)
    blob16 = nc.declare_dram_parameter("blob16", [128, nb16], BF, False)
    out = nc.declare_dram_parameter("out", [S, DOUT], F32, True)

    with TileContext(nc) as tc:
        with (
            tc.tile_pool(name="const", bufs=1) as cp,
            tc.tile_pool(name="work", bufs=1) as wp,
            tc.tile_pool(name="et", bufs=3) as ep,
            tc.tile_pool(name="ctxs", bufs=2) as xp,
            tc.tile_pool(name="io", bufs=4) as iop,
        ):
            # ---- warm scratch (no DMA dependency) ----
            warm = wp.tile([128, 512], BF, tag="warm")
            nc.gpsimd.memset(warm[:], 0.0)
            warmf = wp.tile([128, 16], F32, tag="warmf")
            nc.gpsimd.memset(warmf[:], 0.0)
            warme = wp.tile([128, 16], F32, tag="warme")
            nc.scalar.activation(warme[:], warmf[:], Exp)   # load exp table

            # ---- input DMAs, spread across queues ----
            # scalar HW queue: the k/v-side activations (first thing PE
            # needs).  gpsimd would be a software DGE at ~90 GB/s — avoid.
            xkv_sb = []
            for p2 in range(4):
                t = cp.tile([128, 2 * m_pad], BF, tag=f"xkv{p2}")
                nc.scalar.dma_start(out=t[:], in_=xkv[p2][:])
                xkv_sb.append(t)
            xq_sb = [None] * 4
            # sync queue: weights, const blobs, second q half
            wkv_sb = cp.tile([128, 2 * NCH * 128], BF, tag="wkv")
            nc.sync.dma_start(out=wkv_sb[:], in_=wkv[:])
            wq_sb = cp.tile([128, NCH * 128], F8, tag="wq")
            nc.sync.dma_start(out=wq_sb[:], in_=wq[:])
            b16_sb = cp.tile([128, nb16], BF, tag="b16")
            nc.sync.dma_start(out=b16_sb[:], in_=blob16[:])
            b32_sb = cp.tile([128, nb32], F32, tag="b32")
            nc.sync.dma_start(out=b32_sb[:], in_=blob32[:])
            for sb in (0, 1, 2, 3):
                t = cp.tile([128, NCH * 512], F8, tag=f"xq{sb}")
                nc.sync.dma_start(out=t[:], in_=xq[sb][:])
                xq_sb[sb] = t

            wk_sb = wkv_sb[:, 0:NCH * 128]
            wv_sb = wkv_sb[:, NCH * 128:2 * NCH * 128]
            bq_sb = b32_sb[:, 0:1]
            bk_sb = b32_sb[:, 1:2]
            bvT_sb = b32_sb[:, 2:3]
            mcolf_sb = b32_sb[:, 3:3 + nkt]
            identf_sb = b32_sb[:, 3 + nkt:3 + nkt + 128]
            mcol_sb = b16_sb[:, 0:nkt]
            identb_sb = b16_sb[:, nkt:nkt + 128]

            qT_sb = wp.tile([128, S], BF, tag="qT")
            kT_sb = wp.tile([128, m_pad], BF, tag="kT")
            vT_sb = wp.tile([128, m_pad], BF, tag="vT")
            v_sb = wp.tile([128, m_pad], BF, tag="v")

            def xkv_ap(c, o, w):
                return xkv_sb[c // 2][:, (c % 2) * m_pad + o:
                                      (c % 2) * m_pad + o + w]

            # ================= prologue =================
            with (
                tc.tile_pool(name="warmp", bufs=1, space="PSUM") as wpp,
                tc.tile_pool(name="proj", bufs=3, space="PSUM") as pp,
                tc.tile_pool(name="vtp", bufs=2, space="PSUM") as vp,
            ):
                # PE warm-up: engage HAM while DMAs stream
                wps = wpp.tile([128, 512], F32, tag="warmps")
                for _ in range(4):
                    nc.tensor.matmul(wps[:], warm[:, 0:128], warm[:],
                                     start=True, stop=True)

                # ---- kT[d, j'] projection (DMA-paced; warm MMs between) ----
                kps = [pp.tile([128, w], F32, tag="proj", name=f"kps{i}")
                       for i, (o, w) in enumerate(kblk)]
                for c in range(NCH):
                    for i, (o, w) in enumerate(kblk):
                        nc.tensor.matmul(
                            kps[i][:], wk_sb[:, c * 128:(c + 1) * 128],
                            xkv_ap(c, o, w),
                            start=(c == 0), stop=(c == NCH - 1))
                    nc.tensor.matmul(wps[:], warm[:, 0:128], warm[:],
                                     start=True, stop=True)
                for i, (o, w) in enumerate(kblk):
                    nc.scalar.activation(kT_sb[:, o:o + w], kps[i][:], Ident,
                                         bias=bk_sb)

                # ---- qT first half (i in [0, 1024)) ----
                qps = [pp.tile([128, 512], F32, tag="proj", name=f"qps{i}")
                       for i in range(2)]
                for c in range(NCH):
                    for sb in range(2):
                        nc.tensor.matmul(
                            qps[sb][:], wq_sb[:, c * 128:(c + 1) * 128],
                            xq_sb[sb][:, c * 512:(c + 1) * 512],
                            start=(c == 0), stop=(c == NCH - 1))
                for sb in range(2):
                    nc.scalar.activation(
                        qT_sb[:, sb * 512:(sb + 1) * 512], qps[sb][:], Ident,
                        bias=bq_sb, scale=1.0 / WSCALE)

                # ---- vT then transpose to natural v, mask-zero pads ----
                vps = [pp.tile([128, w], F32, tag="proj", name=f"vps{i}")
                       for i, (o, w) in enumerate(kblk)]
                for c in range(NCH):
                    for i, (o, w) in enumerate(kblk):
                        nc.tensor.matmul(
                            vps[i][:], wv_sb[:, c * 128:(c + 1) * 128],
                            xkv_ap(c, o, w),
                            start=(c == 0), stop=(c == NCH - 1))
                for i, (o, w) in enumerate(kblk):
                    nc.vector.tensor_scalar_add(
                        vT_sb[:, o:o + w], vps[i][:], bvT_sb)
                for jt in range(nkt):
                    tv = vp.tile([128, 128], BF, tag="vt", name=f"vt{jt}")
                    nc.tensor.transpose(
                        tv[:], vT_sb[:, jt * 128:(jt + 1) * 128], identb_sb)
                    nc.vector.tensor_scalar_mul(
                        v_sb[:, jt * 128:(jt + 1) * 128], tv[:],
                        mcolf_sb[:, jt:jt + 1])

            # ================= middle sweep =================
            with (
                tc.tile_pool(name="sp", bufs=2, space="PSUM") as sp,
                tc.tile_pool(name="cpp", bufs=1, space="PSUM") as cpp,
                tc.tile_pool(name="sums", bufs=1, space="PSUM") as smp,
                tc.tile_pool(name="tp", bufs=1, space="PSUM") as tpp,
            ):
                sums_ps = smp.tile([128, 512], F32, tag="sums")
                ss_sb = wp.tile([128, 512], F32, tag="ss")
                rt_sb = wp.tile([128, 2 * 8], F32, tag="rt")
                recip_sb = wp.tile([128, 2 * 8], F32, tag="recip")

                def qh1_chunk(sb):
                    # second-half q projection, borrowing the tp bank;
                    # bias copy on DVE so ACT keeps streaming exps
                    qp = tpp.tile([128, 512], F32, tag="tp", name=f"qh1_{sb}")
                    for c in range(NCH):
                        nc.tensor.matmul(
                            qp[:], wq_sb[:, c * 128:(c + 1) * 128],
                            xq_sb[sb][:, c * 512:(c + 1) * 512],
                            start=(c == 0), stop=(c == NCH - 1))
                    nc.vector.tensor_scalar(
                        qT_sb[:, sb * 512:(sb + 1) * 512], qp[:],
                        1.0 / WSCALE, bq_sb, MUL, ADD)

                def emit_scores(h, jt):
                    ps = sp.tile([128, 1024], F32, tag="sc", name=f"sc{h}_{jt}")
                    for n in range(2):
                        nc.tensor.matmul(
                            ps[:, n * 512:(n + 1) * 512],
                            kT_sb[:, jt * 128:(jt + 1) * 128],
                            qT_sb[:, h * 1024 + n * 512:
                                  h * 1024 + (n + 1) * 512],
                            start=True, stop=True)
                    et = ep.tile([128, 1024], BF, tag="et", name=f"et{h}_{jt}")
                    nc.scalar.activation(et[:], ps[:], Exp, scale=SCALE)
                    return et

                def emit_consume(h, jt, et, ctx_ps):
                    for icg in range(2):
                        base = 64 * h + 32 * icg
                        nc.tensor.matmul(
                            sums_ps[base:base + 1, :],
                            mcol_sb[:, jt:jt + 1],
                            et[:, icg * 512:(icg + 1) * 512],
                            start=(jt == 0), stop=(jt == nkt - 1),
                            tile_position=(0, base),
                            skip_group_check=True)
                    for n in range(2):
                        nc.tensor.matmul(
                            ctx_ps[:, n * 512:(n + 1) * 512],
                            v_sb[:, jt * 128:(jt + 1) * 128],
                            et[:, n * 512:(n + 1) * 512],
                            start=(jt == 0), stop=(jt == nkt - 1))

                def tail(h, ctx_ps):
                    # transposes go to the tp bank for h=0 (sweep still needs
                    # sp), and to freed sp slots for h=1 (no bank reuse stalls)
                    if h == 0:
                        tquads = [(tpp.tile([128, 512], F32, tag="tp",
                                            name="t0a"), 0)] * 3
                        tquads = [tquads[0], tquads[1], tquads[2]]
                    else:
                        ta = sp.tile([128, 1024], F32, tag="sc", name="t1a")
                        tb = sp.tile([128, 1024], F32, tag="sc", name="t1b")
                        tquads = [(ta, 0), (ta, 4), (tb, 0)]

                    # denominators: sums psum rows {64h, 64h+32}
                    nc.vector.tensor_copy(
                        ss_sb[64 * h:64 * h + 64, :],
                        sums_ps[64 * h:64 * h + 64, :])
                    ctx_sb = xp.tile([128, 1024], F32, tag="ctxT",
                                     name=f"ctxT{h}")
                    nc.vector.tensor_copy(ctx_sb[:, 0:512], ctx_ps[:, 0:512])

                    stile, soff = tquads[0]
                    for b in range(4):
                        tcol = (soff + b) * 128
                        nc.tensor.transpose(
                            stile[:, tcol:tcol + 128],
                            ss_sb[:, b * 128:(b + 1) * 128], identf_sb)
                        for icg in range(2):
                            it = h * 8 + icg * 4 + b
                            src = tcol + 64 * h + 32 * icg
                            nc.vector.tensor_copy(
                                rt_sb[:, it:it + 1], stile[:, src:src + 1])
                    nc.vector.reciprocal(
                        recip_sb[:, h * 8:(h + 1) * 8],
                        rt_sb[:, h * 8:(h + 1) * 8])

                    nc.vector.tensor_copy(ctx_sb[:, 512:1024],
                                          ctx_ps[:, 512:1024])
                    for icg in range(2):
                        ctile, coff = tquads[1 + icg]
                        o_sb = iop.tile([128, 512], F32, tag="osb",
                                        name=f"osb{h}_{icg}")
                        for b in range(4):
                            tcol = (coff + b) * 128
                            nc.tensor.transpose(
                                ctile[:, tcol:tcol + 128],
                                ctx_sb[:, icg * 512 + b * 128:
                                       icg * 512 + (b + 1) * 128], identf_sb)
                            it = h * 8 + icg * 4 + b
                            if h == 1 and b % 2 == 0:
                                nc.scalar.activation(
                                    o_sb[:, b * 128:(b + 1) * 128],
                                    ctile[:, tcol:tcol + 128], Copy,
                                    scale=recip_sb[:, it:it + 1])
                            else:
                                nc.vector.tensor_scalar_mul(
                                    o_sb[:, b * 128:(b + 1) * 128],
                                    ctile[:, tcol:tcol + 128],
                                    recip_sb[:, it:it + 1])
                        r0 = h * 1024 + icg * 512
                        eng = nc.scalar if (h == 1 and icg == 1) else nc.sync
                        eng.dma_start(
                            out=out[r0:r0 + 512, :].rearrange(
                                "(b p) d -> p b d", b=4),
                            in_=o_sb[:])

                ctx_tiles = {}

                def ctx_tile(h):
                    if h not in ctx_tiles:
                        ctx_tiles[h] = cpp.tile([128, 1024], F32, tag="ctx",
                                                name=f"ctx{h}")
                    return ctx_tiles[h]

                for h in range(2):
                    prev = None
                    for jt in range(nkt):
                        et = emit_scores(h, jt)
                        if h == 1 and jt == 1:
                            # h0 tail here: its PE transposes hide in the
                            # exp pipeline-fill bubble of the h1 sweep
                            tail(0, ctx_tile(0))
                        if prev is not None:
                            emit_consume(h, jt - 1, prev, ctx_tile(h))
                        prev = et
                        if h == 0 and jt == 2:
                            qh1_chunk(2)
                        elif h == 0 and jt == 4:
                            qh1_chunk(3)
                    emit_consume(h, nkt - 1, prev, ctx_tile(h))
                tail(1, ctx_tile(1))

    nc.compile()
    return nc


def _prep_core_inputs(xb, Wq, bq, Wk, bk, Wv, bv, maskb, m_pad):
    """Host-side layout prep for one batch element."""
    nkt = m_pad // 128
    idx = np.nonzero(maskb != 0)[0]
    cnt = len(idx)

    xg = np.zeros((m_pad, DIN), dtype=np.float32)
    xg[:cnt] = xb[idx]

    def chunkT(m, dtp):  # [rows, DIN] -> [128, NCH*rows], m.T chunked over DIN
        mt = np.ascontiguousarray(m.T)          # [DIN, rows]
        c = mt.shape[1]
        return np.ascontiguousarray(
            mt.reshape(NCH, 128, c).transpose(1, 0, 2).reshape(128, NCH * c)
        ).astype(dtp)

    xt = xb.T.reshape(NCH, 128, 4, 512)          # [c, p, sb, s']
    xq4 = np.ascontiguousarray(
        xt.transpose(1, 2, 0, 3).reshape(128, 4 * NCH * 512)).astype(FP8)

    xkvt = xg.T.reshape(NCH, 128, m_pad)         # [c, p, j']
    m01 = np.zeros((m_pad,), dtype=np.float32)
    m01[:cnt] = 1.0
    mt = np.ascontiguousarray(m01.reshape(nkt, 128).T)   # [128, nkt]

    identf = np.eye(128, dtype=np.float32)
    blob32 = np.concatenate([
        bq.reshape(128, 1), bk.reshape(128, 1), bv.reshape(128, 1),
        mt, identf], axis=1).astype(np.float32)
    blob16 = np.concatenate([mt, identf], axis=1).astype(BF16)

    d = {
        "wq": chunkT(Wq * WSCALE, FP8),
        "wkv": np.concatenate([chunkT(Wk, BF16), chunkT(Wv, BF16)], axis=1),
        "blob32": np.ascontiguousarray(blob32),
        "blob16": np.ascontiguousarray(blob16),
    }
    for sb in range(4):
        d[f"xq{sb}"] = np.ascontiguousarray(
            xq4[:, sb * NCH * 512:(sb + 1) * NCH * 512])
    for p2 in range(4):
        d[f"xkv{p2}"] = np.ascontiguousarray(
            np.concatenate([xkvt[2 * p2], xkvt[2 * p2 + 1]], axis=1)
        ).astype(BF16)
    return d


def kernel(x, Wq, bq, Wk, bk, Wv, bv, attention_mask, _trace=False):
    from concourse.bass_utils import run_bass_kernel_spmd

    x = np.asarray(x, dtype=np.float32)
    Wq = np.asarray(Wq, dtype=np.float32)
    Wk = np.asarray(Wk, dtype=np.float32)
    Wv = np.asarray(Wv, dtype=np.float32)
    bq = np.asarray(bq, dtype=np.float32)
    bk = np.asarray(bk, dtype=np.float32)
    bv = np.asarray(bv, dtype=np.float32)
    mask = np.asarray(attention_mask)

    counts = [int((mask[b, 0] != 0).sum()) for b in range(B)]
    m_pad = min(S, max(512, int(-(-max(counts) // 128) * 128)))

    if ("nc", m_pad) not in _CACHED:
        _CACHED[("nc", m_pad)] = _build(m_pad)
    nc = _CACHED[("nc", m_pad)]

    in_maps = [
        _prep_core_inputs(x[b], Wq, bq, Wk, bk, Wv, bv, mask[b, 0], m_pad)
        for b in range(B)
    ]
    res = run_bass_kernel_spmd(
        nc, in_maps, core_ids=list(range(N_CORES)), trace=_trace)
    out = np.stack([res.results[b]["out"] for b in range(B)]).astype(np.float32)
    if _trace:
        _CACHED["exec_time_ns"] = res.exec_time_ns
    return out


# revision 12
# speedup vs baseline: 1.6695x; 1.1168x over previous
"""Single-head attention on 8 TRN2 NeuronCores — data-parallel over batch.

Reference (per batch element b):
    q = x @ Wq.T + bq; k = x @ Wk.T + bk; v = x @ Wv.T + bv     [S, D]
    scores = q @ k.T / sqrt(S); masked where attention_mask==0
    out = softmax(scores) @ v                                    [S, D]

Shapes: B=8, S=2048, DIN=1024, D=128.  Core i computes batch element i.

Key optimizations over the straightforward version:
  * Key compaction: softmax is permutation-invariant over keys and ~half the
    keys are masked out.  Host-side we gather the unmasked keys and pad to a
    fixed M_pad (multiple of 128).  scores/exp/context then run on M_pad keys
    instead of S.  Padded keys get v=0 (zeroed via mask multiply) and are
    excluded from the softmax denominator by using the 0/1 mask column as the
    stationary operand of the denominator matmuls.
  * fp8 (e4m3) x and 16*Wq for the q projection halve that DMA; the 1/16
    rescale is folded into the bias-apply copy.  The k/v path stays bf16 —
    the near-uniform softmax averages ~1000 v rows with heavy cancellation,
    so fp8 noise in v does not shrink relative to the output.
  * Input DMAs are spread across the sync/gpsimd/vector/scalar queues and
    sized large, since each dma_start costs ~0.7us of issue time on its queue.
  * The middle loop is split into two query halves (i in [0,1024), [1024,2048))
    so the context accumulator fits in 2 PSUM banks and the first half's
    normalize/store tail overlaps the second half's sweep.  Consumer matmuls
    (denominator + context) of step jt are emitted after scores of step jt+1
    so the in-order PE never idles waiting for exp.
  * Denominator matmuls (M=1) for the 2 query chunks of a half are packed into
    different 32-column groups of the PE array via tile_position, sharing one
    PSUM bank across all 4 (half, chunk) pairs.
  * PE warm-up matmuls during the initial DMA engage the HAM clock-gate
    (1.2 -> 2.4 GHz) before the real work starts; more are interleaved into
    the DMA-paced k projection to keep it engaged.
"""

import numpy as np
import ml_dtypes

B, S, DIN, DOUT = 8, 2048, 1024, 128
N_CORES = 8
NCH = DIN // 128          # 8 contraction chunks
BF16 = ml_dtypes.bfloat16
FP8 = ml_dtypes.float8_e4m3fn
SCALE = 1.0 / float(np.sqrt(S))
WSCALE = 16.0             # Wq shipped as 16*Wq in fp8; undone in bias copy

_CACHED = {}


def _blocks(total, maxw=512):
    out = []
    off = 0
    while off < total:
        w = min(maxw, total - off)
        out.append((off, w))
        off += w
    return out


def _build(m_pad):
    import concourse.bacc as bacc
    import concourse.mybir as mybir
    from concourse.tile import TileContext

    dt = mybir.dt
    F32, BF, F8 = dt.float32, dt.bfloat16, dt.float8e4
    Exp = mybir.ActivationFunctionType.Exp
    Ident = mybir.ActivationFunctionType.Identity
    Copy = mybir.ActivationFunctionType.Copy
    MUL, ADD = mybir.AluOpType.mult, mybir.AluOpType.add

    nkt = m_pad // 128            # key tiles
    kblk = _blocks(m_pad)         # column blocks for the k/v projections
    nb32 = 3 + nkt + 128          # blob32: bq | bk | bvT | mcolf | identf
    nb16 = nkt + 128              # blob16: mcol | identb

    nc = bacc.Bacc("TRN2", target_bir_lowering=False)

    # xq[p, ((sb*8+c)*512)+s'] = x[sb*512+s', c*128+p], 4 slabs of s
    xq = [nc.declare_dram_parameter(f"xq{sb}", [128, NCH * 512], F8, False)
          for sb in range(4)]
    # xkv2[p2][p, i*m_pad + j'] = x[idx[j'], (2*p2+i)*128+p]
    xkv = [nc.declare_dram_parameter(f"xkv{p2}", [128, 2 * m_pad], BF, False)
           for p2 in range(4)]
    wkv = nc.declare_dram_parameter("wkv", [128, 2 * NCH * 128], BF, False)
    wq = nc.declare_dram_parameter("wq", [128, NCH * 128], F8, False)
    blob32 = nc.declare_dram_parameter("blob32", [128, nb32], F32, False)
    blob16 = nc.declare_dram_parameter("blob16", [128, nb16], BF, False)
    out = nc.declare_dram_parameter("out", [S, DOUT], F32, True)

    with TileContext(nc) as tc:
        with (
            tc.tile_pool(name="const", bufs=1) as cp,
            tc.tile_pool(name="work", bufs=1) as wp,
            tc.tile_pool(name="et", bufs=3) as ep,
            tc.tile_pool(name="ctxs", bufs=2) as xp,
            tc.tile_pool(name="io", bufs=4) as iop,
        ):
            # ---- warm scratch (no DMA dependency) ----
            warm = wp.tile([128, 512], BF, tag="warm")
            nc.gpsimd.memset(warm[:], 0.0)
            warmf = wp.tile([128, 16], F32, tag="warmf")
            nc.gpsimd.memset(warmf[:], 0.0)
            warme = wp.tile([128, 16], F32, tag="warme")
            nc.scalar.activation(warme[:], warmf[:], Exp)   # load exp table

            # ---- input DMAs, spread across queues ----
            # Both HW DGE queues (sync + scalar) pull the k/v-side
            # activations first, interleaved in the order the k projection
            # consumes them; q-side and weights follow on sync.
            xkv_sb = [None] * 4
            for p2, eng in ((0, nc.scalar), (None, None), (2, nc.scalar)):
                if p2 is None:
                    # sync queue start: weights first, then its xkv share
                    wkv_sb = cp.tile([128, 2 * NCH * 128], BF, tag="wkv")
                    nc.sync.dma_start(out=wkv_sb[:], in_=wkv[:])
                    t = cp.tile([128, 2 * m_pad], BF, tag="xkv1")
                    nc.sync.dma_start(out=t[:], in_=xkv[1][:])
                    xkv_sb[1] = t
                    continue
                t = cp.tile([128, 2 * m_pad], BF, tag=f"xkv{p2}")
                eng.dma_start(out=t[:], in_=xkv[p2][:])
                xkv_sb[p2] = t
            b32_sb = cp.tile([128, nb32], F32, tag="b32")
            nc.sync.dma_start(out=b32_sb[:], in_=blob32[:])
            b16_sb = cp.tile([128, nb16], BF, tag="b16")
            nc.sync.dma_start(out=b16_sb[:], in_=blob16[:])
            t = cp.tile([128, 2 * m_pad], BF, tag="xkv3")
            nc.sync.dma_start(out=t[:], in_=xkv[3][:])
            xkv_sb[3] = t
            wq_sb = cp.tile([128, NCH * 128], F8, tag="wq")
            nc.sync.dma_start(out=wq_sb[:], in_=wq[:])
            xq_sb = [None] * 4
            for sb in (0, 1, 2, 3):
                t = cp.tile([128, NCH * 512], F8, tag=f"xq{sb}")
                nc.sync.dma_start(out=t[:], in_=xq[sb][:])
                xq_sb[sb] = t

            wk_sb = wkv_sb[:, 0:NCH * 128]
            wv_sb = wkv_sb[:, NCH * 128:2 * NCH * 128]
            bq_sb = b32_sb[:, 0:1]
            bk_sb = b32_sb[:, 1:2]
            bvT_sb = b32_sb[:, 2:3]
            mcolf_sb = b32_sb[:, 3:3 + nkt]
            identf_sb = b32_sb[:, 3 + nkt:3 + nkt + 128]
            mcol_sb = b16_sb[:, 0:nkt]
            identb_sb = b16_sb[:, nkt:nkt + 128]

            qT_sb = wp.tile([128, S], BF, tag="qT")
            kT_sb = wp.tile([128, m_pad], BF, tag="kT")
            vT_sb = wp.tile([128, m_pad], BF, tag="vT")
            v_sb = wp.tile([128, m_pad], BF, tag="v")

            def xkv_ap(c, o, w):
                return xkv_sb[c // 2][:, (c % 2) * m_pad + o:
                                      (c % 2) * m_pad + o + w]

            # ================= prologue =================
            with (
                tc.tile_pool(name="warmp", bufs=1, space="PSUM") as wpp,
                tc.tile_pool(name="proj", bufs=3, space="PSUM") as pp,
                tc.tile_pool(name="vtp", bufs=2, space="PSUM") as vp,
            ):
                # PE warm-up: engage HAM while DMAs stream
                wps = wpp.tile([128, 512], F32, tag="warmps")
                for _ in range(4):
                    nc.tensor.matmul(wps[:], warm[:, 0:128], warm[:],
                                     start=True, stop=True)

                # ---- kT[d, j'] projection (DMA-paced; warm MMs between) ----
                kps = [pp.tile([128, w], F32, tag="proj", name=f"kps{i}")
                       for i, (o, w) in enumerate(kblk)]
                for c in range(NCH):
                    for i, (o, w) in enumerate(kblk):
                        nc.tensor.matmul(
                            kps[i][:], wk_sb[:, c * 128:(c + 1) * 128],
                            xkv_ap(c, o, w),
                            start=(c == 0), stop=(c == NCH - 1))
                    nc.tensor.matmul(wps[:], warm[:, 0:128], warm[:],
                                     start=True, stop=True)
                for i, (o, w) in enumerate(kblk):
                    nc.scalar.activation(kT_sb[:, o:o + w], kps[i][:], Ident,
                                         bias=bk_sb)

                # ---- vT then transpose to natural v, mask-zero pads ----
                vps = [pp.tile([128, w], F32, tag="proj", name=f"vps{i}")
                       for i, (o, w) in enumerate(kblk)]
                for c in range(NCH):
                    for i, (o, w) in enumerate(kblk):
                        nc.tensor.matmul(
                            vps[i][:], wv_sb[:, c * 128:(c + 1) * 128],
                            xkv_ap(c, o, w),
                            start=(c == 0), stop=(c == NCH - 1))
                for i, (o, w) in enumerate(kblk):
                    nc.vector.tensor_scalar_add(
                        vT_sb[:, o:o + w], vps[i][:], bvT_sb)
                for jt in range(nkt):
                    tv = vp.tile([128, 128], BF, tag="vt", name=f"vt{jt}")
                    nc.tensor.transpose(
                        tv[:], vT_sb[:, jt * 128:(jt + 1) * 128], identb_sb)
                    nc.vector.tensor_scalar_mul(
                        v_sb[:, jt * 128:(jt + 1) * 128], tv[:],
                        mcolf_sb[:, jt:jt + 1])

                # ---- qT first half (i in [0, 1024)) ----
                qps = [pp.tile([128, 512], F32, tag="proj", name=f"qps{i}")
                       for i in range(2)]
                for c in range(NCH):
                    for sb in range(2):
                        nc.tensor.matmul(
                            qps[sb][:], wq_sb[:, c * 128:(c + 1) * 128],
                            xq_sb[sb][:, c * 512:(c + 1) * 512],
                            start=(c == 0), stop=(c == NCH - 1))
                for sb in range(2):
                    nc.scalar.activation(
                        qT_sb[:, sb * 512:(sb + 1) * 512], qps[sb][:], Ident,
                        bias=bq_sb, scale=1.0 / WSCALE)


            # ================= middle sweep =================
            with (
                tc.tile_pool(name="sp", bufs=2, space="PSUM") as sp,
                tc.tile_pool(name="cpp", bufs=1, space="PSUM") as cpp,
                tc.tile_pool(name="sums", bufs=1, space="PSUM") as smp,
                tc.tile_pool(name="tp", bufs=1, space="PSUM") as tpp,
            ):
                sums_ps = smp.tile([128, 512], F32, tag="sums")
                ss_sb = wp.tile([128, 512], F32, tag="ss")
                rt_sb = wp.tile([128, 2 * 8], F32, tag="rt")
                recip_sb = wp.tile([128, 2 * 8], F32, tag="recip")

                qh1_state = {}

                def qh1_piece(sb, cpair):
                    # second-half q projection, 2 chunks at a time in the tp
                    # bank; bias copy on DVE so ACT keeps streaming exps
                    if sb not in qh1_state:
                        qh1_state[sb] = tpp.tile([128, 512], F32, tag="tp",
                                                 name=f"qh1_{sb}")
                    qp = qh1_state[sb]
                    for c in (2 * cpair, 2 * cpair + 1):
                        nc.tensor.matmul(
                            qp[:], wq_sb[:, c * 128:(c + 1) * 128],
                            xq_sb[sb][:, c * 512:(c + 1) * 512],
                            start=(c == 0), stop=(c == NCH - 1))
                    if cpair == 3:
                        nc.vector.tensor_scalar(
                            qT_sb[:, sb * 512:(sb + 1) * 512], qp[:],
                            1.0 / WSCALE, bq_sb, MUL, ADD)

                def emit_scores(h, jt):
                    ps = sp.tile([128, 1024], F32, tag="sc", name=f"sc{h}_{jt}")
                    for n in range(2):
                        nc.tensor.matmul(
                            ps[:, n * 512:(n + 1) * 512],
                            kT_sb[:, jt * 128:(jt + 1) * 128],
                            qT_sb[:, h * 1024 + n * 512:
                                  h * 1024 + (n + 1) * 512],
                            start=True, stop=True)
                    et = ep.tile([128, 1024], BF, tag="et", name=f"et{h}_{jt}")
                    nc.scalar.activation(et[:], ps[:], Exp, scale=SCALE)
                    return et

                def emit_consume(h, jt, et, ctx_ps):
                    for icg in range(2):
                        base = 64 * h + 32 * icg
                        nc.tensor.matmul(
                            sums_ps[base:base + 1, :],
                            mcol_sb[:, jt:jt + 1],
                            et[:, icg * 512:(icg + 1) * 512],
                            start=(jt == 0), stop=(jt == nkt - 1),
                            tile_position=(0, base),
                            skip_group_check=True)
                    for n in range(2):
                        nc.tensor.matmul(
                            ctx_ps[:, n * 512:(n + 1) * 512],
                            v_sb[:, jt * 128:(jt + 1) * 128],
                            et[:, n * 512:(n + 1) * 512],
                            start=(jt == 0), stop=(jt == nkt - 1))

                tail_state = {}

                def tail_stage(h, ctx_ps, stage):
                    """Stage 0: denominators; 1: first 512 ctx cols out;
                    2: second 512.  Staged so tail(0) spreads over h1 steps."""
                    st = tail_state.setdefault(h, {})
                    if stage == 0:
                        if h == 0:
                            st['t'] = [(tpp.tile([128, 512], F32, tag="tp",
                                                 name="t0a"), 0)] * 3
                        else:
                            ta = sp.tile([128, 1024], F32, tag="sc", name="t1a")
                            tb = sp.tile([128, 1024], F32, tag="sc", name="t1b")
                            st['t'] = [(ta, 0), (ta, 4), (tb, 0)]
                        nc.vector.tensor_copy(
                            ss_sb[64 * h:64 * h + 64, :],
                            sums_ps[64 * h:64 * h + 64, :])
                        ctx_sb = xp.tile([128, 1024], F32, tag="ctxT",
                                         name=f"ctxT{h}")
                        st['ctx'] = ctx_sb
                        nc.vector.tensor_copy(ctx_sb[:, 0:512],
                                              ctx_ps[:, 0:512])
                        stile, soff = st['t'][0]
                        for b in range(4):
                            tcol = (soff + b) * 128
                            nc.tensor.transpose(
                                stile[:, tcol:tcol + 128],
                                ss_sb[:, b * 128:(b + 1) * 128], identf_sb)
                            for icg in range(2):
                                it = h * 8 + icg * 4 + b
                                srcc = tcol + 64 * h + 32 * icg
                                nc.vector.tensor_copy(
                                    rt_sb[:, it:it + 1],
                                    stile[:, srcc:srcc + 1])
                        nc.vector.reciprocal(
                            recip_sb[:, h * 8:(h + 1) * 8],
                            rt_sb[:, h * 8:(h + 1) * 8])
                        nc.vector.tensor_copy(st['ctx'][:, 512:1024],
                                              ctx_ps[:, 512:1024])
                        return
                    icg = stage - 1
                    ctx_sb = st['ctx']
                    ctile, coff = st['t'][1 + icg]
                    o_sb = iop.tile([128, 512], F32, tag="osb",
                                    name=f"osb{h}_{icg}")
                    for b in range(4):
                        tcol = (coff + b) * 128
                        nc.tensor.transpose(
                            ctile[:, tcol:tcol + 128],
                            ctx_sb[:, icg * 512 + b * 128:
                                   icg * 512 + (b + 1) * 128], identf_sb)
                        it = h * 8 + icg * 4 + b
                        if h == 1 and b % 2 == 0:
                            nc.scalar.activation(
                                o_sb[:, b * 128:(b + 1) * 128],
                                ctile[:, tcol:tcol + 128], Copy,
                                scale=recip_sb[:, it:it + 1])
                        else:
                            nc.vector.tensor_scalar_mul(
                                o_sb[:, b * 128:(b + 1) * 128],
                                ctile[:, tcol:tcol + 128],
                                recip_sb[:, it:it + 1])
                    r0 = h * 1024 + icg * 512
                    eng = nc.scalar if (h == 1 and icg == 1) else nc.sync
                    eng.dma_start(
                        out=out[r0:r0 + 512, :].rearrange(
                            "(b p) d -> p b d", b=4),
                        in_=o_sb[:])

                ctx_tiles = {}

                def ctx_tile(h):
                    if h not in ctx_tiles:
                        ctx_tiles[h] = cpp.tile([128, 1024], F32, tag="ctx",
                                                name=f"ctx{h}")
                    return ctx_tiles[h]

                for h in range(2):
                    prev = None
                    for jt in range(nkt):
                        et = emit_scores(h, jt)
                        if h == 1 and jt in (1, 2, 3):
                            # h0 tail staged here: its PE work hides in the
                            # exp pipeline-fill bubble of the h1 sweep
                            tail_stage(0, ctx_tile(0), jt - 1)
                        if prev is not None:
                            emit_consume(h, jt - 1, prev, ctx_tile(h))
                        prev = et
                        if h == 0 and 1 <= jt <= 4:
                            qh1_piece(2, jt - 1)
                        elif h == 0 and 5 <= jt <= 8:
                            qh1_piece(3, jt - 5)
                    emit_consume(h, nkt - 1, prev, ctx_tile(h))
                for stage in range(3):
                    tail_stage(1, ctx_tile(1), stage)

    nc.compile()
    return nc


def _prep_core_inputs(xb, Wq, bq, Wk, bk, Wv, bv, maskb, m_pad):
    """Host-side layout prep for one batch element."""
    nkt = m_pad // 128
    idx = np.nonzero(maskb != 0)[0]
    cnt = len(idx)

    xg = np.zeros((m_pad, DIN), dtype=np.float32)
    xg[:cnt] = xb[idx]

    def chunkT(m, dtp):  # [rows, DIN] -> [128, NCH*rows], m.T chunked over DIN
        mt = np.ascontiguousarray(m.T)          # [DIN, rows]
        c = mt.shape[1]
        return np.ascontiguousarray(
            mt.reshape(NCH, 128, c).transpose(1, 0, 2).reshape(128, NCH * c)
        ).astype(dtp)

    xt = xb.T.reshape(NCH, 128, 4, 512)          # [c, p, sb, s']
    xq4 = np.ascontiguousarray(
        xt.transpose(1, 2, 0, 3).reshape(128, 4 * NCH * 512)).astype(FP8)

    xkvt = xg.T.reshape(NCH, 128, m_pad)         # [c, p, j']
    m01 = np.zeros((m_pad,), dtype=np.float32)
    m01[:cnt] = 1.0
    mt = np.ascontiguousarray(m01.reshape(nkt, 128).T)   # [128, nkt]

    identf = np.eye(128, dtype=np.float32)
    blob32 = np.concatenate([
        bq.reshape(128, 1), bk.reshape(128, 1), bv.reshape(128, 1),
        mt, identf], axis=1).astype(np.float32)
    blob16 = np.concatenate([mt, identf], axis=1).astype(BF16)

    d = {
        "wq": chunkT(Wq * WSCALE, FP8),
        "wkv": np.concatenate([chunkT(Wk, BF16), chunkT(Wv, BF16)], axis=1),
        "blob32": np.ascontiguousarray(blob32),
        "blob16": np.ascontiguousarray(blob16),
    }
    for sb in range(4):
        d[f"xq{sb}"] = np.ascontiguousarray(
            xq4[:, sb * NCH * 512:(sb + 1) * NCH * 512])
    for p2 in range(4):
        d[f"xkv{p2}"] = np.ascontiguousarray(
            np.concatenate([xkvt[2 * p2], xkvt[2 * p2 + 1]], axis=1)
        ).astype(BF16)
    return d


def kernel(x, Wq, bq, Wk, bk, Wv, bv, attention_mask, _trace=False):
    from concourse.bass_utils import run_bass_kernel_spmd

    x = np.asarray(x, dtype=np.float32)
    Wq = np.asarray(Wq, dtype=np.float32)
    Wk = np.asarray(Wk, dtype=np.float32)
    Wv = np.asarray(Wv, dtype=np.float32)
    bq = np.asarray(bq, dtype=np.float32)
    bk = np.asarray(bk, dtype=np.float32)
    bv = np.asarray(bv, dtype=np.float32)
    mask = np.asarray(attention_mask)

    counts = [int((mask[b, 0] != 0).sum()) for b in range(B)]
    m_pad = min(S, max(512, int(-(-max(counts) // 128) * 128)))

    if ("nc", m_pad) not in _CACHED:
        _CACHED[("nc", m_pad)] = _build(m_pad)
    nc = _CACHED[("nc", m_pad)]

    in_maps = [
        _prep_core_inputs(x[b], Wq, bq, Wk, bk, Wv, bv, mask[b, 0], m_pad)
        for b in range(B)
    ]
    res = run_bass_kernel_spmd(
        nc, in_maps, core_ids=list(range(N_CORES)), trace=_trace)
    out = np.stack([res.results[b]["out"] for b in range(B)]).astype(np.float32)
    if _trace:
        _CACHED["exec_time_ns"] = res.exec_time_ns
    return out
